# revision 1
# baseline (speedup 1.0000x reference)
"""Trainium2 Bass kernel for nn_MultiHeadAttention_377957122345.

B=16, T=512, C=1024, H=16, D=64.  Data-parallel over batch: each of the
8 NeuronCores computes attention for 2 sequences; no collectives.

Per-core device program (SPMD, identical on all cores):
  - inputs staged on host as transposed layouts: xT [C, 1024] (c_in on
    partitions), W^T [c_in, c_out] for all four projections, and
    rel_pos_bias with the causal mask folded in (-1e30 above diagonal).
  - all matmuls in bf16 with fp32 PSUM accumulation; softmax in fp32.
  - Q/K projections produce q^T/k^T (head_dim on partitions); V and the
    final output are produced in natural [t, c] layout.
  - causal block skipping: for query block i only key blocks j <= i are
    computed, masked entries never touched.
  - scores S = q^T.T @ k^T land in PSUM [128 t, w s]; bias added in
    PSUM; exp on ScalarE with row-sum accumulation; P normalized and
    transposed 128x128 via TensorE so the AV matmul can contract over s.
"""

import numpy as np

B, T, C, H = 16, 512, 1024, 16
D = C // H  # 64
N_CORES = 8
B_LOC = B // N_CORES  # 2 sequences per core
NT = B_LOC * T  # 1024 tokens per core
P = 128
KO = C // P  # 8 contraction subtiles
TB = T // P  # 4 query blocks per sequence
NEG = -1e30

_CACHE = {}

import contextlib


def _nullcm():
    return contextlib.nullcontext()



def _split_big_waits(nc, mybir, limit=1):
    # This walrus build rejects instructions whose sync_info.on_wait
    # exceeds its slot count (the Tile end-of-kernel Drain trips it).
    # Move excess waits onto dedicated same-engine NOPs placed directly
    # before the instruction; the engine stalls on those first, so the
    # semantics are unchanged.
    f = nc.m.functions[0]
    for bb in f.blocks:
        out = []
        changed = False
        for inst in bb.instructions:
            si = getattr(inst, "sync_info", None)
            waits = list(si.on_wait) if si is not None and si.on_wait else []
            if len(waits) > limit:
                changed = True
                head, tail = waits[:-limit], waits[-limit:]
                for k in range(0, len(head), limit):
                    out.append(
                        mybir.InstNoOp(
                            name=f"I-{nc.next_id()}",
                            sync_info=mybir.SyncInfo(
                                on_wait=head[k : k + limit], on_update=[]
                            ),
                            bass_nofuse=True,
                            engine=inst.engine,
                        )
                    )
                si.on_wait = tail
            out.append(inst)
        if changed:
            bb.instructions = out
    return nc


def build_program(split_waits=True, reps=1, skip_attn=False, skip_proj=False, dbg=False, bias_dma_psum=False, attn_lvl=3):
    import concourse.bass as bass
    import concourse.mybir as mybir
    import concourse.tile as tile

    fp32 = mybir.dt.float32
    bf16 = mybir.dt.bfloat16
    Act = mybir.ActivationFunctionType

    nc = bass.Bass()
    xT = nc.dram_tensor("xT", [C, NT], fp32, kind="ExternalInput")
    wqT = nc.dram_tensor("wqT", [C, C], fp32, kind="ExternalInput")
    wkT = nc.dram_tensor("wkT", [C, C], fp32, kind="ExternalInput")
    wvT = nc.dram_tensor("wvT", [C, C], fp32, kind="ExternalInput")
    woT = nc.dram_tensor("woT", [C, C], fp32, kind="ExternalInput")
    bqd = nc.dram_tensor("bq", [C], fp32, kind="ExternalInput")
    bkd = nc.dram_tensor("bk", [C], fp32, kind="ExternalInput")
    bvd = nc.dram_tensor("bv", [C], fp32, kind="ExternalInput")
    bod = nc.dram_tensor("bo", [C], fp32, kind="ExternalInput")
    biasm = nc.dram_tensor("biasm", [H, T, T], bf16, kind="ExternalInput")
    y = nc.dram_tensor("y", [NT, C], fp32, kind="ExternalOutput")

    with tile.TileContext(nc) as tc, \
         tc.tile_pool(name="consts", bufs=1) as consts, \
         tc.tile_pool(name="stage", bufs=2) as stage, \
         tc.tile_pool(name="persist", bufs=1) as persist, \
         tc.tile_pool(name="biasp", bufs=4) as biasp, \
         tc.tile_pool(name="soft", bufs=3) as soft, \
         tc.tile_pool(name="small", bufs=8) as small, \
         tc.tile_pool(name="ypool", bufs=3) as ypool, \
         tc.tile_pool(name="psA", bufs=5, space="PSUM") as psA, \
         tc.tile_pool(name="psO", bufs=3, space="PSUM") as psO, \
         (tc.For_i(0, reps, 1) if reps > 1 else _nullcm()):

        # ----- constants -----
        bq_sb = consts.tile([P, KO], fp32, name="bq_sb")
        nc.sync.dma_start(out=bq_sb, in_=bqd.rearrange("(o p) -> p o", p=P))
        bk_sb = consts.tile([P, KO], fp32, name="bk_sb")
        nc.sync.dma_start(out=bk_sb, in_=bkd.rearrange("(o p) -> p o", p=P))
        bv_sb = consts.tile([P, C], fp32, name="bv_sb")
        bv_ap = bvd[:]
        nc.sync.dma_start(
            out=bv_sb,
            in_=bass.AP(tensor=bv_ap.tensor, offset=bv_ap.offset, ap=[[0, P], [1, C]]),
        )
        bo_sb = consts.tile([P, C], fp32, name="bo_sb")
        bo_ap = bod[:]
        nc.sync.dma_start(
            out=bo_sb,
            in_=bass.AP(tensor=bo_ap.tensor, offset=bo_ap.offset, ap=[[0, P], [1, C]]),
        )

        # ----- load inputs, cast to bf16 -----
        def load_cast(dram, name):
            t_bf = persist.tile([P, KO, C], bf16, name=name)
            for ko in range(KO):
                st = stage.tile([P, C], fp32, tag="stage")
                nc.sync.dma_start(out=st, in_=dram[ko * P : (ko + 1) * P, :])
                nc.any.tensor_copy(out=t_bf[:, ko, :], in_=st)
            return t_bf

        xT_bf = load_cast(xT, "xT_bf")
        wq_bf = load_cast(wqT, "wq_bf")
        wk_bf = load_cast(wkT, "wk_bf")
        wv_bf = load_cast(wvT, "wv_bf")
        wo_bf = load_cast(woT, "wo_bf")

        qT_bf = persist.tile([P, KO, NT], bf16, name="qT_bf")
        kT_bf = persist.tile([P, KO, NT], bf16, name="kT_bf")
        vaug = persist.tile([P, NT // P, H * (D + 1)], bf16, name="vaug")
        oT_bf = persist.tile([P, KO, NT], bf16, name="oT_bf")

        NCH = NT // 512  # 2 free-dim chunks of 512

        # ----- Q/K projections -> transposed layout [c_out on partitions, t]
        # q is pre-scaled by 1/sqrt(D) (bq comes pre-scaled from host).
        if skip_proj:
            nc.any.memset(qT_bf[:], 0.0)
            nc.any.memset(kT_bf[:], 0.0)
            nc.any.memset(vv_bf[:], 0.0)
        for w_bf, out_bf, b_sb, scl in (() if skip_proj else (
            (wq_bf, qT_bf, bq_sb, 1.0 / np.sqrt(D)),
            (wk_bf, kT_bf, bk_sb, 1.0),
        )):
            for mo in range(KO):
                for nch in range(NCH):
                    ps = psA.tile([P, 512], fp32, tag="psA")
                    for ko in range(KO):
                        nc.tensor.matmul(
                            ps,
                            lhsT=w_bf[:, ko, mo * P : (mo + 1) * P],
                            rhs=xT_bf[:, ko, nch * 512 : (nch + 1) * 512],
                            start=(ko == 0),
                            stop=(ko == KO - 1),
                        )
                    nc.scalar.activation(
                        out=out_bf[:, mo, nch * 512 : (nch + 1) * 512],
                        in_=ps,
                        func=Act.Identity,
                        bias=b_sb[:, mo : mo + 1],
                        scale=scl,
                    )

        # ----- V projection -> ones-augmented layout: head h occupies
        # columns [h*65, h*65+64) with a ones column at h*65+64, so the AV
        # matmul emits the softmax denominator as PSUM row 64.
        nc.any.memset(vaug[:], 1.0)
        for to in range(NT // P if not skip_proj else 0):
            for nch in range(NCH):
                ps = psA.tile([P, 512], fp32, tag="psA")
                for ko in range(KO):
                    nc.tensor.matmul(
                        ps,
                        lhsT=xT_bf[:, ko, to * P : (to + 1) * P],
                        rhs=wv_bf[:, ko, nch * 512 : (nch + 1) * 512],
                        start=(ko == 0),
                        stop=(ko == KO - 1),
                    )
                for hh in range(8):
                    h = nch * 8 + hh
                    nc.any.tensor_add(
                        out=vaug[:, to, h * (D + 1) : h * (D + 1) + D],
                        in0=ps[:, hh * D : (hh + 1) * D],
                        in1=bv_sb[:, h * D : (h + 1) * D],
                    )

        r_dram = nc.dram_tensor("r_scratch", [H, NT], fp32)
        rd = r_dram[:]

        # ----- attention (S computed transposed: [s on partitions, t]) --
        if skip_attn or attn_lvl < 3:
            nc.any.memset(oT_bf[:], 0.0)
        for b in range(B_LOC if not skip_attn else 0):
            for h in range(H):
                po = (h % 2) * D  # partition offset of this head's dims
                mo = h // 2  # which 128-block of c holds this head pair
                qh = qT_bf[po : po + D, mo, b * T : (b + 1) * T]  # [64, 512]
                kh = kT_bf[po : po + D, mo, b * T : (b + 1) * T]  # [64, 512]
                # S^T_j = k_j @ q^T for t >= j*128 (causal), +bias, exp.
                # Unnormalized probabilities; denominator comes from the
                # ones column in vaug during AV.
                PTs = []
                for j in range(TB):
                    wj = T - j * P
                    psS = psA.tile([P, 512], fp32, tag="psA")
                    nc.tensor.matmul(
                        psS[:, :wj],
                        lhsT=kh[:, j * P : (j + 1) * P],
                        rhs=qh[:, j * P :],
                        start=True,
                        stop=True,
                    )
                    expb_sb = biasp.tile([P, 512], bf16, tag="bias", bufs=5)
                    nc.sync.dma_start(
                        out=expb_sb[:, :wj],
                        in_=biasm[h, j * P : (j + 1) * P, j * P :],
                    )
                    PT0 = soft.tile([P, 512], bf16, tag="PT0", bufs=6)
                    nc.scalar.activation(
                        out=PT0[:, :wj], in_=psS[:, :wj], func=Act.Exp
                    )
                    PT = soft.tile([P, 512], bf16, tag="PT", bufs=8)
                    nc.any.tensor_mul(
                        out=PT[:, :wj], in0=PT0[:, :wj], in1=expb_sb[:, :wj]
                    )
                    PTs.append(PT)
                if attn_lvl < 2:
                    continue
                r_sb = small.tile([1, T], fp32, tag="r", bufs=3)
                for i in range(TB):
                    pst = psO.tile([D + 1, P], fp32, tag="psO")
                    for j in range(i + 1):
                        nc.tensor.matmul(
                            pst,
                            lhsT=vaug[:, b * TB + j, h * (D + 1) : (h + 1) * (D + 1)],
                            rhs=PTs[j][:, (i - j) * P : (i - j + 1) * P],
                            start=(j == 0),
                            stop=(j == i),
                        )
                    nc.vector.reciprocal(
                        out=r_sb[0:1, i * P : (i + 1) * P],
                        in_=pst[D : D + 1, :],
                    )
                    nc.any.tensor_copy(
                        out=oT_bf[po : po + D, mo, b * T + i * P : b * T + (i + 1) * P],
                        in_=pst[:D, :],
                    )
                nc.sync.dma_start(
                    out=r_dram[h, b * T : (b + 1) * T], in_=r_sb[0:1, :]
                )

        # ---- batched softmax normalization: broadcast the packed 1/l
        # DRAM scratch into the oT layout with two replicating DMAs (DRAM
        # sources allow step-0 dims), then one in-place multiply.
        if not skip_attn and attn_lvl >= 3:
            r_bc = soft.tile([P, KO, NT // 2], fp32, name="r_bc", bufs=1)
            for half in range(2):
                for tch in range(2):
                    nc.sync.dma_start(
                        out=r_bc[half * D : (half + 1) * D],
                        in_=bass.AP(
                            tensor=rd.tensor,
                            offset=rd.offset + half * NT + tch * (NT // 2),
                            ap=[[0, D], [2 * NT, KO], [1, NT // 2]],
                        ),
                    )
                    nc.any.tensor_mul(
                        out=oT_bf[
                            half * D : (half + 1) * D,
                            :,
                            tch * (NT // 2) : (tch + 1) * (NT // 2),
                        ],
                        in0=oT_bf[
                            half * D : (half + 1) * D,
                            :,
                            tch * (NT // 2) : (tch + 1) * (NT // 2),
                        ],
                        in1=r_bc[half * D : (half + 1) * D],
                    )

        if dbg:
            for nm, tl in (("qT_dbg", qT_bf), ("kT_dbg", kT_bf),
                           ("vv_dbg", vv_bf), ("oT_dbg", oT_bf)):
                dt_ = nc.dram_tensor(nm, list(tl.shape), bf16, kind="ExternalOutput")
                nc.sync.dma_start(out=dt_[:], in_=tl[:])

        # ----- output projection -> y [t, c_out] fp32
        for to in range(NT // P):
            for nch in range(NCH):
                ps = psA.tile([P, 512], fp32, tag="psA")
                for co in range(KO):
                    nc.tensor.matmul(
                        ps,
                        lhsT=oT_bf[:, co, to * P : (to + 1) * P],
                        rhs=wo_bf[:, co, nch * 512 : (nch + 1) * 512],
                        start=(co == 0),
                        stop=(co == KO - 1),
                    )
                ysb = ypool.tile([P, 512], fp32, tag="y")
                nc.any.tensor_add(
                    out=ysb, in0=ps, in1=bo_sb[:, nch * 512 : (nch + 1) * 512]
                )
                nc.sync.dma_start(
                    out=y[to * P : (to + 1) * P, nch * 512 : (nch + 1) * 512],
                    in_=ysb,
                )

    if split_waits:
        _split_big_waits(nc, mybir, limit=1)
    return nc


def make_in_maps(inputs):
    x = np.ascontiguousarray(np.asarray(inputs["x"], dtype=np.float32))
    wT = {
        k: np.ascontiguousarray(np.asarray(inputs[f"W{k}"], dtype=np.float32).T)
        for k in "qkvo"
    }
    bq = np.asarray(inputs["bq"], dtype=np.float32) * np.float32(1.0 / np.sqrt(D))
    bk = np.asarray(inputs["bk"], dtype=np.float32)
    bv = np.asarray(inputs["bv"], dtype=np.float32)
    bo = np.asarray(inputs["bo"], dtype=np.float32)
    import ml_dtypes

    bm = np.asarray(inputs["rel_pos_bias"], dtype=np.float32)[:, :T, :T].copy()
    iu = np.triu_indices(T, 1)
    bm[:, iu[0], iu[1]] = NEG
    # multiplicative form: exp(S+bias) = exp(S) * exp(bias); causal mask
    # becomes an exact multiplicative zero. Transposed to [h, s, t].
    bm = np.ascontiguousarray(
        np.exp(bm.transpose(0, 2, 1)).astype(ml_dtypes.bfloat16)
    )

    xT_all = x.reshape(N_CORES, NT, C).transpose(0, 2, 1)
    in_maps = []
    for c in range(N_CORES):
        in_maps.append(
            {
                "xT": np.ascontiguousarray(xT_all[c]),
                "wqT": wT["q"],
                "wkT": wT["k"],
                "wvT": wT["v"],
                "woT": wT["o"],
                "bq": bq,
                "bk": bk,
                "bv": bv,
                "bo": bo,
                "biasm": bm,
            }
        )
    return in_maps


def build_jitted(nc, n_cores=N_CORES):
    """Build a persistent jitted shard_map executable for `nc` (the
    multi-core path of bass2jax.run_bass_via_pjrt, kept resident so repeat
    kernel() calls skip retracing)."""
    import jax
    from jax.experimental.shard_map import shard_map
    from jax.sharding import Mesh, NamedSharding, PartitionSpec

    from concourse import mybir
    from concourse.bass2jax import (
        _bass_exec_p,
        install_neuronx_cc_hook,
        partition_id_tensor,
    )

    install_neuronx_cc_hook()
    partition_name = nc.partition_id_tensor.name if nc.partition_id_tensor else None

    in_names, out_names, out_avals, zero_outs = [], [], [], []
    for alloc in nc.m.functions[0].allocations:
        if not isinstance(alloc, mybir.MemoryLocationSet):
            continue
        name = alloc.memorylocations[0].name
        if alloc.kind == "ExternalInput":
            if name != partition_name:
                in_names.append(name)
        elif alloc.kind == "ExternalOutput":
            out_names.append(name)
            shape = tuple(alloc.tensor_shape)
            dtype = mybir.dt.np(alloc.dtype)
            out_avals.append(jax.core.ShapedArray(shape, dtype))
            zero_outs.append(np.zeros(shape, dtype))
    n_params = len(in_names)
    n_outs = len(out_avals)
    all_in_names = list(in_names) + list(out_names)
    if partition_name is not None:
        all_in_names.append(partition_name)
    donate = tuple(range(n_params, n_params + n_outs))

    def _body(*args):
        operands = list(args)
        if partition_name is not None:
            operands.append(partition_id_tensor())
        outs = _bass_exec_p.bind(
            *operands,
            out_avals=tuple(out_avals),
            in_names=tuple(all_in_names),
            out_names=tuple(out_names),
            lowering_input_output_aliases=(),
            sim_require_finite=True,
            sim_require_nnan=True,
            nc=nc,
        )
        return tuple(outs)

    devices = jax.devices()[:n_cores]
    mesh = Mesh(np.asarray(devices), ("core",))
    in_specs = (PartitionSpec("core"),) * (n_params + n_outs)
    out_specs = (PartitionSpec("core"),) * n_outs
    jitted = jax.jit(
        shard_map(_body, mesh=mesh, in_specs=in_specs, out_specs=out_specs,
                  check_rep=False),
        donate_argnums=donate,
        keep_unused=True,
    )
    sharding = NamedSharding(mesh, PartitionSpec("core"))
    return jitted, in_names, out_names, out_avals, zero_outs, sharding


def get_runner():
    """Build the program + executable once; return in_maps -> per-core
    output dicts."""
    if "runner" in _CACHE:
        return _CACHE["runner"]
    import jax

    nc = build_program()
    jitted, in_names, out_names, out_avals, zero_outs, sharding = build_jitted(nc)
    n_cores = N_CORES

    def runner(in_maps):
        concat_in = [
            jax.device_put(
                np.concatenate(
                    [np.asarray(in_maps[c][nm]) for c in range(n_cores)], axis=0
                ),
                sharding,
            )
            for nm in in_names
        ]
        zeros = [
            jax.device_put(
                np.zeros((n_cores * z.shape[0], *z.shape[1:]), z.dtype), sharding
            )
            for z in zero_outs
        ]
        out_arrs = jitted(*concat_in, *zeros)
        return [
            {
                nm: np.asarray(out_arrs[i]).reshape(n_cores, *out_avals[i].shape)[c]
                for i, nm in enumerate(out_names)
            }
            for c in range(n_cores)
        ]

    _CACHE["runner"] = runner
    _CACHE["nc"] = nc
    return runner


def kernel(**inputs) -> np.ndarray:
    runner = get_runner()
    in_maps = make_in_maps(inputs)
    results = runner(in_maps)
    out = np.concatenate(
        [results[c]["y"].reshape(B_LOC, T, C) for c in range(N_CORES)], axis=0
    )
    return out.astype(np.float32)



# revision 22
# speedup vs baseline: 1.0716x; 1.0716x over previous
"""Trainium2 Bass kernel for nn_MultiHeadAttention_377957122345.

B=16, T=512, C=1024, H=16, D=64.  Data-parallel over batch: each of the
8 NeuronCores computes attention for 2 sequences; no collectives.

Per-core device program (SPMD, identical on all cores):
  - all large inputs staged on host as bf16 in transposed layouts:
    xT [C, NT] (c_in on partitions), W^T [c_in, c_out] for all four
    projections (wq pre-scaled by 1/sqrt(D)), and the rel-pos bias in
    multiplicative form exp(bias) with the causal mask folded in as exact
    zeros, packed per head into the 10 needed causal blocks [H, 128, 1280].
  - V bias is folded into the output-projection bias on the host
    (bo_eff = bo + Wo @ bv), so the V path is a pure matmul.
  - all matmuls bf16 with fp32 PSUM accumulation.
  - scores S^T = k^T.T @ q^T land transposed [s, t]; per head the four
    causal blocks occupy 3 PSUM banks (j=0 | j=1,j=3 packed | j=2) so
    exp + bias-mult take 3 instructions each instead of 4.
  - AV contracts over s with a ones-augmented V layout; PSUM row 64 of
    the [65, 512] AV tile is the softmax denominator.  Normalization:
    reciprocal -> rank-1 matmul broadcasts 1/l into rows 64:128 of the
    same PSUM bank -> one elementwise multiply writes normalized oT.
  - output projection computes y^T [c_out, t] so bo is a per-partition
    activation bias; host un-transposes.
"""

import contextlib

import numpy as np

B, T, C, H = 16, 512, 1024, 16
D = C // H  # 64
N_CORES = 8
B_LOC = B // N_CORES  # 2 sequences per core
NT = B_LOC * T  # 1024 tokens per core
P = 128
KO = C // P  # 8 contraction subtiles
TB = T // P  # 4 query blocks per sequence
NEG = -1e30
# packed causal-block column offsets per head: j=0 | j=1 | j=3 | j=2
BIAS_OFF = {0: 0, 1: 512, 3: 896, 2: 1024}
BIAS_W = 1280

_CACHE = {}


def _nullcm():
    return contextlib.nullcontext()


def _split_big_waits(nc, mybir, limit=1):
    # This walrus build rejects instructions whose sync_info.on_wait
    # exceeds its slot count (the Tile end-of-kernel Drain trips it).
    # Move excess waits onto dedicated same-engine NOPs placed directly
    # before the instruction; the engine stalls on those first, so the
    # semantics are unchanged.
    f = nc.m.functions[0]
    for bb in f.blocks:
        out = []
        changed = False
        for inst in bb.instructions:
            si = getattr(inst, "sync_info", None)
            waits = list(si.on_wait) if si is not None and si.on_wait else []
            if len(waits) > limit:
                changed = True
                head, tail = waits[:-limit], waits[-limit:]
                for k in range(0, len(head), limit):
                    out.append(
                        mybir.InstNoOp(
                            name=f"I-{nc.next_id()}",
                            sync_info=mybir.SyncInfo(
                                on_wait=head[k : k + limit], on_update=[]
                            ),
                            bass_nofuse=True,
                            engine=inst.engine,
                        )
                    )
                si.on_wait = tail
            out.append(inst)
        if changed:
            bb.instructions = out
    return nc


def build_program(split_waits=True, reps=1):
    import concourse.bass as bass
    import concourse.mybir as mybir
    import concourse.tile as tile

    fp32 = mybir.dt.float32
    bf16 = mybir.dt.bfloat16
    Act = mybir.ActivationFunctionType

    nc = bass.Bass()
    xT = nc.dram_tensor("xT", [C, NT], bf16, kind="ExternalInput")
    wqT = nc.dram_tensor("wqT", [C, C], bf16, kind="ExternalInput")
    wkT = nc.dram_tensor("wkT", [C, C], bf16, kind="ExternalInput")
    wvT = nc.dram_tensor("wvT", [C, C], bf16, kind="ExternalInput")
    woT = nc.dram_tensor("woT", [C, C], bf16, kind="ExternalInput")
    bqd = nc.dram_tensor("bq", [C], fp32, kind="ExternalInput")
    bkd = nc.dram_tensor("bk", [C], fp32, kind="ExternalInput")
    bod = nc.dram_tensor("bo", [C], fp32, kind="ExternalInput")
    expbd = nc.dram_tensor("expb", [H, P, BIAS_W], bf16, kind="ExternalInput")
    yT = nc.dram_tensor("yT", [C, NT], bf16, kind="ExternalOutput")

    with tile.TileContext(nc) as tc, \
         tc.tile_pool(name="consts", bufs=1) as consts, \
         tc.tile_pool(name="persist", bufs=1) as persist, \
         tc.tile_pool(name="biask", bufs=1) as biask, \
         tc.tile_pool(name="soft", bufs=3) as soft, \
         tc.tile_pool(name="small", bufs=2) as small, \
         tc.tile_pool(name="ypool", bufs=3) as ypool, \
         tc.tile_pool(name="psA", bufs=6, space="PSUM") as psA, \
         tc.tile_pool(name="psO", bufs=2, space="PSUM") as psO:

        # ----- prologue (outside the timing loop): first wo load.  Each
        # loop iteration re-loads wo at its *end* for the next iteration —
        # out-proj reads wo until the very end of an iteration, so a
        # top-of-body load would stall whichever queue carries it (and
        # every later DMA on that queue) until the previous iteration
        # fully drains.
        wo_bf = persist.tile([P, KO, C], bf16, name="wo_bf")
        wo_src = woT.rearrange("(o p) c -> p o c", p=P)
        nc.sync.dma_start(out=wo_bf, in_=wo_src)

        # static tile allocations, shared by every unrolled body copy
        bq_sb = consts.tile([P, KO], fp32, name="bq_sb")
        bk_sb = consts.tile([P, KO], fp32, name="bk_sb")
        bo_sb = consts.tile([P, KO], fp32, name="bo_sb")
        ones_sb = consts.tile([1, D], bf16, name="ones_sb")
        xT_bf = persist.tile([P, KO, C], bf16, name="xT_bf")
        wq_bf = persist.tile([P, KO, C], bf16, name="wq_bf")
        wk_bf = persist.tile([P, KO, C], bf16, name="wk_bf")
        wv_bf = persist.tile([P, KO, C], bf16, name="wv_bf")
        qT_bf = persist.tile([P, KO, NT], bf16, name="qT_bf")
        kT_bf = persist.tile([P, KO, NT], bf16, name="kT_bf")
        vaug = persist.tile([P, NT // P, H * (D + 1)], bf16, name="vaug")
        oT_bf = persist.tile([P, KO, NT], bf16, name="oT_bf")
        vaug_hx = vaug.rearrange("p t (h x) -> p t h x", x=D + 1)

        x_src = xT.rearrange("(o p) c -> p o c", p=P)
        wq_src = wqT.rearrange("(o p) c -> p o c", p=P)
        wk_src = wkT.rearrange("(o p) c -> p o c", p=P)
        wv_src = wvT.rearrange("(o p) c -> p o c", p=P)

        NCH = NT // 512  # 2 free-dim chunks of 512

        def emit_loads():
            nc.sync.dma_start(out=bq_sb, in_=bqd.rearrange("(o p) -> p o", p=P))
            nc.sync.dma_start(out=bk_sb, in_=bkd.rearrange("(o p) -> p o", p=P))
            nc.sync.dma_start(out=bo_sb, in_=bod.rearrange("(o p) -> p o", p=P))
            nc.vector.memset(ones_sb[:], 1.0)
            # x and wq arrive as per-ko chunks, interleaved, so the first
            # Q-proj matmuls can start early instead of waiting for 4MB
            for ko in range(KO):
                nc.sync.dma_start(out=xT_bf[:, ko], in_=x_src[:, ko])
                nc.sync.dma_start(out=wq_bf[:, ko], in_=wq_src[:, ko])
            nc.sync.dma_start(out=wk_bf, in_=wk_src)
            nc.sync.dma_start(out=wv_bf, in_=wv_src)
            # ones column for each head (AV emits the softmax denominator)
            nc.gpsimd.memset(vaug_hx[:, :, :, D : D + 1], 1.0)

        # ----- Q/K projections -> [c_out on partitions, t] (bias via ACT;
        # wq/bq pre-scaled by 1/sqrt(D) on host)
        def qk_proj(mo):
            for w_bf, out_bf, b_sb in (
                (wq_bf, qT_bf, bq_sb),
                (wk_bf, kT_bf, bk_sb),
            ):
                for nch in range(NCH):
                    ps = psA.tile([P, 512], fp32, tag="psA")
                    for ko in range(KO):
                        nc.tensor.matmul(
                            ps,
                            lhsT=w_bf[:, ko, mo * P : (mo + 1) * P],
                            rhs=xT_bf[:, ko, nch * 512 : (nch + 1) * 512],
                            start=(ko == 0),
                            stop=(ko == KO - 1),
                        )
                    nc.scalar.activation(
                        out=out_bf[:, mo, nch * 512 : (nch + 1) * 512],
                        in_=ps,
                        func=Act.Identity,
                        bias=b_sb[:, mo : mo + 1],
                    )

        # ----- V projection -> ones-augmented layout (no bias: folded into bo)
        def v_proj(to):
            for nch in range(NCH):
                ps = psA.tile([P, 512], fp32, tag="psA")
                for ko in range(KO):
                    nc.tensor.matmul(
                        ps,
                        lhsT=xT_bf[:, ko, to * P : (to + 1) * P],
                        rhs=wv_bf[:, ko, nch * 512 : (nch + 1) * 512],
                        start=(ko == 0),
                        stop=(ko == KO - 1),
                    )
                nc.vector.tensor_copy(
                    out=vaug_hx[:, to, nch * 8 : (nch + 1) * 8, 0:D],
                    in_=ps.rearrange("p (h d) -> p h d", d=D),
                )

        # ----- attention for one (seq, head), split into an S half and an
        # AV half so S(h+1) can be emitted before AV(h) (PE runs in order;
        # this keeps it busy while ACT/DVE chew on exp+mask of head h) -----
        def attn_s(b, h):
            po = (h % 2) * D
            mo = h // 2
            qh = qT_bf[po : po + D, mo, b * T : (b + 1) * T]  # [64, 512]
            kh = kT_bf[po : po + D, mo, b * T : (b + 1) * T]
            # just-in-time bias load; deep ring (12) so the SP queue drains
            # well before the iteration ends, letting the next iteration's
            # x/wq loads prefetch across the loop boundary
            ebt = biask.tile([P, BIAS_W], bf16, tag="biask", bufs=12, name="ebt")
            nc.sync.dma_start(out=ebt, in_=expbd[h])
            # S^T blocks: j=0 -> bank A [512]; j=1,3 -> bank B [384|128];
            # j=2 -> bank C [256]
            psS0 = psA.tile([P, 512], fp32, tag="psA")
            nc.tensor.matmul(
                psS0, lhsT=kh[:, 0:P], rhs=qh[:, 0:], start=True, stop=True
            )
            psS13 = psA.tile([P, 512], fp32, tag="psA")
            nc.tensor.matmul(
                psS13[:, 0:384], lhsT=kh[:, P : 2 * P], rhs=qh[:, P:],
                start=True, stop=True,
            )
            nc.tensor.matmul(
                psS13[:, 384:512], lhsT=kh[:, 3 * P : 4 * P], rhs=qh[:, 3 * P :],
                start=True, stop=True,
            )
            psS2 = psA.tile([P, 512], fp32, tag="psA")
            nc.tensor.matmul(
                psS2[:, 0:256], lhsT=kh[:, 2 * P : 3 * P], rhs=qh[:, 2 * P :],
                start=True, stop=True,
            )
            # exp (unnormalized) then multiplicative bias+mask
            PTa = soft.tile([P, 512], bf16, tag="PTa", bufs=3)
            PTb = soft.tile([P, 512], bf16, tag="PTb", bufs=3)
            PTc = soft.tile([P, 256], bf16, tag="PTc", bufs=3)
            for ps_in, pt, w, off in (
                (psS0, PTa, 512, 0),
                (psS13, PTb, 512, 512),
                (psS2, PTc, 256, 1024),
            ):
                pt0 = soft.tile([P, 512], bf16, tag="PT0", bufs=3)
                nc.scalar.activation(out=pt0[:, :w], in_=ps_in[:, :w], func=Act.Exp)
                nc.vector.tensor_mul(
                    out=pt[:, :w], in0=pt0[:, :w], in1=ebt[:, off : off + w]
                )
            return PTa, PTb, PTc

        def attn_av(b, h, pts):
            PTa, PTb, PTc = pts
            po = (h % 2) * D
            mo = h // 2
            # AV: psOt rows 0:64 = unnormalized out, row 64 = denominator l
            psOt = psO.tile([P, 512], fp32, tag="psO")

            def pt_rhs(i, j):
                if j == 0:
                    return PTa[:, i * P : (i + 1) * P]
                if j == 1:
                    return PTb[:, (i - 1) * P : i * P]
                if j == 3:
                    return PTb[:, 384:512]
                return PTc[:, (i - 2) * P : (i - 1) * P]

            for i in range(TB):
                for j in range(i + 1):
                    nc.tensor.matmul(
                        psOt[0 : D + 1, i * P : (i + 1) * P],
                        lhsT=vaug[:, b * TB + j, h * (D + 1) : (h + 1) * (D + 1)],
                        rhs=pt_rhs(i, j),
                        start=(j == 0),
                        stop=(j == i),
                    )
            # normalize: r = 1/l; rank-1 matmul broadcasts r into rows 64:128
            # of the same PSUM bank; ACT stages it to SBUF (DVE tensor ops
            # cannot read two PSUM operands); one multiply writes oT
            r_sb = small.tile([1, T], bf16, tag="r", bufs=2)
            with nc.allow_low_precision(reason="bf16 softmax denominators"):
                nc.vector.reciprocal(out=r_sb, in_=psOt[D : D + 1, :])
            nc.tensor.matmul(
                psOt[D : 2 * D, :], lhsT=ones_sb, rhs=r_sb, start=True, stop=True
            )
            rb = soft.tile([D, T], bf16, tag="rb", bufs=2)
            nc.scalar.activation(out=rb, in_=psOt[D : 2 * D, :], func=Act.Copy)
            nc.vector.tensor_mul(
                out=oT_bf[po : po + D, mo, b * T : (b + 1) * T],
                in0=psOt[0:D, :],
                in1=rb,
            )

        # ----- output projection -> y^T [c_out, t] (bias per-partition) ---
        def out_proj(tch, co, defer_store=None):
            ps = psA.tile([P, 512], fp32, tag="psA")
            for ko in range(KO):
                nc.tensor.matmul(
                    ps,
                    lhsT=wo_bf[:, ko, co * P : (co + 1) * P],
                    rhs=oT_bf[:, ko, tch * 512 : (tch + 1) * 512],
                    start=(ko == 0),
                    stop=(ko == KO - 1),
                )
            if defer_store is not None:
                ysb = ypool.tile([P, 512], bf16, tag="ydef", bufs=KO)
            else:
                ysb = ypool.tile([P, 512], bf16, tag="y", bufs=3)
            nc.scalar.activation(
                out=ysb, in_=ps, func=Act.Identity, bias=bo_sb[:, co : co + 1]
            )
            # stores go on the ACT queue (SP must stay clear so the next
            # iteration's input loads can prefetch).  In the final tch=1
            # batch the stores are deferred until all bias-adds have run, so
            # store configs don't delay the PSUM frees that gate the next
            # iteration's first matmuls.
            if defer_store is not None:
                defer_store.append((tch, co, ysb))
            else:
                nc.scalar.dma_start(
                    out=yT[co * P : (co + 1) * P, tch * 512 : (tch + 1) * 512],
                    in_=ysb,
                )

        # ----- one full iteration body: software-pipelined attention (S one
        # head ahead of AV) with V(seq1)/out-proj(seq0) groups interleaved --
        def emit_body():
            emit_loads()
            for mo in range(KO):
                qk_proj(mo)
            for to in range(TB):  # V for seq 0
                v_proj(to)

            # flat schedule of (b, h) attention in order, with filler work
            heads = [(0, h) for h in range(H)] + [(1, h) for h in range(H)]
            filler = {}  # index after which to emit filler group
            for h in range(H):
                if h % 2 == 1 and h // 2 < TB:
                    filler[h] = ("v", TB + h // 2)
                if h % 2 == 1:
                    filler[H + h] = ("o", h // 2)
            pending = None  # (b, h, pts) with S emitted, AV not yet
            for idx, (b, h) in enumerate(heads):
                pts = attn_s(b, h)
                if pending is not None:
                    attn_av(*pending)
                pending = (b, h, pts)
                f = filler.get(idx)
                if f is not None:
                    if f[0] == "v":
                        v_proj(f[1])
                    else:
                        out_proj(0, f[1])
            attn_av(*pending)
            deferred = []
            for co in range(KO):
                out_proj(1, co, defer_store=deferred)
            for tch, co, ysb in deferred:
                nc.scalar.dma_start(
                    out=yT[co * P : (co + 1) * P, tch * 512 : (tch + 1) * 512],
                    in_=ysb,
                )
            # trailing wo load for the next iteration (WAR on this
            # iteration's out-proj reads is satisfied by now)
            nc.scalar.dma_start(out=wo_bf, in_=wo_src)

        # Unroll U iterations per For_i trip: the loop's all-engine drain
        # barrier fires once per U bodies, and within a trip consecutive
        # bodies overlap through the normal tile dependency tracking.
        U = 1
        for cand in (4, 2):
            if reps % cand == 0 and reps >= cand:
                U = cand
                break
        trips = reps // U
        if trips > 1:
            with tc.For_i(0, trips, 1):
                for _ in range(U):
                    emit_body()
        else:
            for _ in range(reps):
                emit_body()

    if split_waits:
        _split_big_waits(nc, mybir, limit=1)
    return nc


def make_in_maps(inputs):
    import ml_dtypes

    bf16 = ml_dtypes.bfloat16
    x = np.asarray(inputs["x"], dtype=np.float32)
    s = np.float32(1.0 / np.sqrt(D))
    wT = {}
    for k in "qkvo":
        w = np.asarray(inputs[f"W{k}"], dtype=np.float32)
        if k == "q":
            w = w * s
        wT[k] = np.ascontiguousarray(w.T).astype(bf16)
    bq = np.asarray(inputs["bq"], dtype=np.float32) * s
    bk = np.asarray(inputs["bk"], dtype=np.float32)
    bo = np.asarray(inputs["bo"], dtype=np.float32) + (
        np.asarray(inputs["Wo"], dtype=np.float32)
        @ np.asarray(inputs["bv"], dtype=np.float32)
    )

    bm = np.asarray(inputs["rel_pos_bias"], dtype=np.float32)[:, :T, :T].copy()
    iu = np.triu_indices(T, 1)
    bm[:, iu[0], iu[1]] = NEG
    # multiplicative form: exp(S+bias) = exp(S) * exp(bias); causal mask
    # becomes an exact multiplicative zero. Transposed to [h, s, t], then
    # the 10 causal blocks packed to [h, 128, 1280] in order j=0,1,3,2.
    bmT = np.exp(bm.transpose(0, 2, 1))
    packed = np.zeros((H, P, BIAS_W), dtype=np.float32)
    for j in range(TB):
        w = T - j * P
        off = BIAS_OFF[j]
        packed[:, :, off : off + w] = bmT[:, j * P : (j + 1) * P, j * P :]
    packed = np.ascontiguousarray(packed.astype(bf16))

    xT_all = x.reshape(N_CORES, NT, C).transpose(0, 2, 1)
    in_maps = []
    for c in range(N_CORES):
        in_maps.append(
            {
                "xT": np.ascontiguousarray(xT_all[c]).astype(bf16),
                "wqT": wT["q"],
                "wkT": wT["k"],
                "wvT": wT["v"],
                "woT": wT["o"],
                "bq": bq,
                "bk": bk,
                "bo": bo,
                "expb": packed,
            }
        )
    return in_maps


def build_jitted(nc, n_cores=N_CORES):
    """Build a persistent jitted shard_map executable for `nc` (the
    multi-core path of bass2jax.run_bass_via_pjrt, kept resident so repeat
    kernel() calls skip retracing)."""
    import jax
    from jax.experimental.shard_map import shard_map
    from jax.sharding import Mesh, NamedSharding, PartitionSpec

    from concourse import mybir
    from concourse.bass2jax import (
        _bass_exec_p,
        install_neuronx_cc_hook,
        partition_id_tensor,
    )

    install_neuronx_cc_hook()
    partition_name = nc.partition_id_tensor.name if nc.partition_id_tensor else None

    in_names, out_names, out_avals, zero_outs = [], [], [], []
    for alloc in nc.m.functions[0].allocations:
        if not isinstance(alloc, mybir.MemoryLocationSet):
            continue
        name = alloc.memorylocations[0].name
        if alloc.kind == "ExternalInput":
            if name != partition_name:
                in_names.append(name)
        elif alloc.kind == "ExternalOutput":
            out_names.append(name)
            shape = tuple(alloc.tensor_shape)
            dtype = mybir.dt.np(alloc.dtype)
            out_avals.append(jax.core.ShapedArray(shape, dtype))
            zero_outs.append(np.zeros(shape, dtype))
    n_params = len(in_names)
    n_outs = len(out_avals)
    all_in_names = list(in_names) + list(out_names)
    if partition_name is not None:
        all_in_names.append(partition_name)
    donate = tuple(range(n_params, n_params + n_outs))

    def _body(*args):
        operands = list(args)
        if partition_name is not None:
            operands.append(partition_id_tensor())
        outs = _bass_exec_p.bind(
            *operands,
            out_avals=tuple(out_avals),
            in_names=tuple(all_in_names),
            out_names=tuple(out_names),
            lowering_input_output_aliases=(),
            sim_require_finite=True,
            sim_require_nnan=True,
            nc=nc,
        )
        return tuple(outs)

    devices = jax.devices()[:n_cores]
    mesh = Mesh(np.asarray(devices), ("core",))
    in_specs = (PartitionSpec("core"),) * (n_params + n_outs)
    out_specs = (PartitionSpec("core"),) * n_outs
    jitted = jax.jit(
        shard_map(_body, mesh=mesh, in_specs=in_specs, out_specs=out_specs,
                  check_rep=False),
        donate_argnums=donate,
        keep_unused=True,
    )
    sharding = NamedSharding(mesh, PartitionSpec("core"))
    return jitted, in_names, out_names, out_avals, zero_outs, sharding


def get_runner():
    """Build the program + executable once; return in_maps -> per-core
    output dicts."""
    if "runner" in _CACHE:
        return _CACHE["runner"]
    import jax

    nc = build_program()
    jitted, in_names, out_names, out_avals, zero_outs, sharding = build_jitted(nc)
    n_cores = N_CORES

    def runner(in_maps):
        concat_in = [
            jax.device_put(
                np.concatenate(
                    [np.asarray(in_maps[c][nm]) for c in range(n_cores)], axis=0
                ),
                sharding,
            )
            for nm in in_names
        ]
        zeros = [
            jax.device_put(
                np.zeros((n_cores * z.shape[0], *z.shape[1:]), z.dtype), sharding
            )
            for z in zero_outs
        ]
        out_arrs = jitted(*concat_in, *zeros)
        return [
            {
                nm: np.asarray(out_arrs[i]).reshape(n_cores, *out_avals[i].shape)[c]
                for i, nm in enumerate(out_names)
            }
            for c in range(n_cores)
        ]

    _CACHE["runner"] = runner
    _CACHE["nc"] = nc
    return runner


def kernel(**inputs) -> np.ndarray:
    runner = get_runner()
    in_maps = make_in_maps(inputs)
    results = runner(in_maps)
    # yT is [C, NT] bf16 per core -> [NT, C] fp32
    out = np.concatenate(
        [
            np.asarray(results[c]["yT"], dtype=np.float32).T.reshape(B_LOC, T, C)
            for c in range(N_CORES)
        ],
        axis=0,
    )
    return out


# revision 24
# speedup vs baseline: 1.5691x; 1.4643x over previous
"""Trainium2 Bass kernel for nn_MultiHeadAttention_377957122345.

B=16, T=512, C=1024, H=16, D=64.  Data-parallel over batch: each of the
8 NeuronCores computes attention for 2 sequences; no collectives.

Per-core device program (SPMD, identical on all cores):
  - all large inputs staged on host as bf16 in transposed layouts:
    xT [C, NT] (c_in on partitions), W^T [c_in, c_out] for all four
    projections (wq pre-scaled by 1/sqrt(D)), and the rel-pos bias in
    multiplicative form exp(bias) with the causal mask folded in as exact
    zeros, packed per head into the 10 needed causal blocks [H, 128, 1280].
  - V bias is folded into the output-projection bias on the host
    (bo_eff = bo + Wo @ bv), so the V path is a pure matmul.
  - all matmuls bf16 with fp32 PSUM accumulation.
  - scores S^T = k^T.T @ q^T land transposed [s, t]; per head the four
    causal blocks occupy 3 PSUM banks (j=0 | j=1,j=3 packed | j=2) so
    exp + bias-mult take 3 instructions each instead of 4.
  - AV contracts over s with a ones-augmented V layout; PSUM row 64 of
    the [65, 512] AV tile is the softmax denominator.  Normalization:
    reciprocal -> rank-1 matmul broadcasts 1/l into rows 64:128 of the
    same PSUM bank -> one elementwise multiply writes normalized oT.
  - output projection computes y^T [c_out, t] so bo is a per-partition
    activation bias; host un-transposes.
"""

import contextlib

import numpy as np

B, T, C, H = 16, 512, 1024, 16
D = C // H  # 64
N_CORES = 8
B_LOC = B // N_CORES  # 2 sequences per core
NT = B_LOC * T  # 1024 tokens per core
P = 128
KO = C // P  # 8 contraction subtiles
TB = T // P  # 4 query blocks per sequence
NEG = -1e30
# packed causal-block column offsets per head: j=0 | j=1 | j=3 | j=2
BIAS_OFF = {0: 0, 1: 512, 3: 896, 2: 1024}
BIAS_W = 1280

_CACHE = {}


def _nullcm():
    return contextlib.nullcontext()


def _split_big_waits(nc, mybir, limit=1):
    # This walrus build rejects instructions whose sync_info.on_wait
    # exceeds its slot count (the Tile end-of-kernel Drain trips it).
    # Move excess waits onto dedicated same-engine NOPs placed directly
    # before the instruction; the engine stalls on those first, so the
    # semantics are unchanged.
    f = nc.m.functions[0]
    for bb in f.blocks:
        out = []
        changed = False
        for inst in bb.instructions:
            si = getattr(inst, "sync_info", None)
            waits = list(si.on_wait) if si is not None and si.on_wait else []
            if len(waits) > limit:
                changed = True
                head, tail = waits[:-limit], waits[-limit:]
                for k in range(0, len(head), limit):
                    out.append(
                        mybir.InstNoOp(
                            name=f"I-{nc.next_id()}",
                            sync_info=mybir.SyncInfo(
                                on_wait=head[k : k + limit], on_update=[]
                            ),
                            bass_nofuse=True,
                            engine=inst.engine,
                        )
                    )
                si.on_wait = tail
            out.append(inst)
        if changed:
            bb.instructions = out
    return nc


def build_program(split_waits=True, reps=1):
    import concourse.bass as bass
    import concourse.mybir as mybir
    import concourse.tile as tile

    fp32 = mybir.dt.float32
    bf16 = mybir.dt.bfloat16
    Act = mybir.ActivationFunctionType

    nc = bass.Bass()
    xT = nc.dram_tensor("xT", [C, NT], bf16, kind="ExternalInput")
    wqT = nc.dram_tensor("wqT", [C, C], bf16, kind="ExternalInput")
    wkT = nc.dram_tensor("wkT", [C, C], bf16, kind="ExternalInput")
    wvT = nc.dram_tensor("wvT", [C, C], bf16, kind="ExternalInput")
    woT = nc.dram_tensor("woT", [C, C], bf16, kind="ExternalInput")
    bqd = nc.dram_tensor("bq", [C], fp32, kind="ExternalInput")
    bkd = nc.dram_tensor("bk", [C], fp32, kind="ExternalInput")
    bod = nc.dram_tensor("bo", [C], fp32, kind="ExternalInput")
    expbd = nc.dram_tensor("expb", [H, P, BIAS_W], bf16, kind="ExternalInput")
    yT = nc.dram_tensor("yT", [C, NT], bf16, kind="ExternalOutput")

    with tile.TileContext(nc) as tc, \
         tc.tile_pool(name="consts", bufs=1) as consts, \
         tc.tile_pool(name="persist", bufs=1) as persist, \
         tc.tile_pool(name="biask", bufs=1) as biask, \
         tc.tile_pool(name="soft", bufs=3) as soft, \
         tc.tile_pool(name="small", bufs=2) as small, \
         tc.tile_pool(name="ypool", bufs=3) as ypool, \
         tc.tile_pool(name="psA", bufs=4, space="PSUM") as psA, \
         tc.tile_pool(name="psO", bufs=2, space="PSUM") as psO:

        wo_bf = persist.tile([P, KO, C], bf16, name="wo_bf")
        wo_src = woT.rearrange("(o p) c -> p o c", p=P)

        # static tile allocations, shared by every unrolled body copy
        bq_sb = consts.tile([P, KO], fp32, name="bq_sb")
        bk_sb = consts.tile([P, KO], fp32, name="bk_sb")
        bo_sb = consts.tile([P, KO], fp32, name="bo_sb")
        ones_sb = consts.tile([1, D], bf16, name="ones_sb")
        xT_bf = persist.tile([P, KO, C], bf16, name="xT_bf")
        wq_bf = persist.tile([P, KO, C], bf16, name="wq_bf")
        wk_bf = persist.tile([P, KO, C], bf16, name="wk_bf")
        wv_bf = persist.tile([P, KO, C], bf16, name="wv_bf")
        qT_bf = persist.tile([P, KO, NT], bf16, name="qT_bf")
        kT_bf = persist.tile([P, KO, NT], bf16, name="kT_bf")
        vaug = persist.tile([P, NT // P, H * (D + 1)], bf16, name="vaug")
        oT_bf = persist.tile([P, KO, NT], bf16, name="oT_bf")
        vaug_hx = vaug.rearrange("p t (h x) -> p t h x", x=D + 1)

        x_src = xT.rearrange("(o p) c -> p o c", p=P)
        wq_src = wqT.rearrange("(o p) c -> p o c", p=P)
        wk_src = wkT.rearrange("(o p) c -> p o c", p=P)
        wv_src = wvT.rearrange("(o p) c -> p o c", p=P)

        NCH = NT // 512  # 2 free-dim chunks of 512

        def emit_loads():
            # wo on the ACT queue: its WAR wait (previous body's out-proj
            # reads) would block the SP queue and every load behind it; the
            # ACT queue has no early-body work to delay
            nc.scalar.dma_start(out=wo_bf, in_=wo_src)
            nc.sync.dma_start(out=bq_sb, in_=bqd.rearrange("(o p) -> p o", p=P))
            nc.sync.dma_start(out=bk_sb, in_=bkd.rearrange("(o p) -> p o", p=P))
            nc.sync.dma_start(out=bo_sb, in_=bod.rearrange("(o p) -> p o", p=P))
            nc.vector.memset(ones_sb[:], 1.0)
            # x and wq arrive as per-ko chunks, interleaved, so the first
            # Q-proj matmuls can start early instead of waiting for 4MB
            for ko in range(KO):
                nc.sync.dma_start(out=xT_bf[:, ko], in_=x_src[:, ko])
                nc.sync.dma_start(out=wq_bf[:, ko], in_=wq_src[:, ko])
            nc.sync.dma_start(out=wk_bf, in_=wk_src)
            nc.sync.dma_start(out=wv_bf, in_=wv_src)
            # ones column for each head (AV emits the softmax denominator)
            nc.gpsimd.memset(vaug_hx[:, :, :, D : D + 1], 1.0)

        # ----- Q/K projections -> [c_out on partitions, t] (bias via ACT;
        # wq/bq pre-scaled by 1/sqrt(D) on host)
        def qk_proj(mo):
            for w_bf, out_bf, b_sb in (
                (wq_bf, qT_bf, bq_sb),
                (wk_bf, kT_bf, bk_sb),
            ):
                for nch in range(NCH):
                    ps = psA.tile([P, 512], fp32, tag="psP", bufs=2)
                    for ko in range(KO):
                        nc.tensor.matmul(
                            ps,
                            lhsT=w_bf[:, ko, mo * P : (mo + 1) * P],
                            rhs=xT_bf[:, ko, nch * 512 : (nch + 1) * 512],
                            start=(ko == 0),
                            stop=(ko == KO - 1),
                        )
                    nc.scalar.activation(
                        out=out_bf[:, mo, nch * 512 : (nch + 1) * 512],
                        in_=ps,
                        func=Act.Identity,
                        bias=b_sb[:, mo : mo + 1],
                    )

        # ----- V projection -> ones-augmented layout (no bias: folded into bo)
        def v_proj(to):
            for nch in range(NCH):
                ps = psA.tile([P, 512], fp32, tag="psP", bufs=2)
                for ko in range(KO):
                    nc.tensor.matmul(
                        ps,
                        lhsT=xT_bf[:, ko, to * P : (to + 1) * P],
                        rhs=wv_bf[:, ko, nch * 512 : (nch + 1) * 512],
                        start=(ko == 0),
                        stop=(ko == KO - 1),
                    )
                nc.vector.tensor_copy(
                    out=vaug_hx[:, to, nch * 8 : (nch + 1) * 8, 0:D],
                    in_=ps.rearrange("p (h d) -> p h d", d=D),
                )

        # ----- attention for one (seq, head), split into an S half and an
        # AV half so S(h+1) can be emitted before AV(h) (PE runs in order;
        # this keeps it busy while ACT/DVE chew on exp+mask of head h) -----
        def attn_s(b, h):
            po = (h % 2) * D
            mo = h // 2
            qh = qT_bf[po : po + D, mo, b * T : (b + 1) * T]  # [64, 512]
            kh = kT_bf[po : po + D, mo, b * T : (b + 1) * T]
            # just-in-time bias load; deep ring (12) so the SP queue drains
            # well before the iteration ends, letting the next iteration's
            # x/wq loads prefetch across the loop boundary
            ebt = biask.tile([P, BIAS_W], bf16, tag="biask", bufs=12, name="ebt")
            nc.sync.dma_start(out=ebt, in_=expbd[h])
            # S^T blocks: j=0 -> bank A [512]; j=1,3 -> bank B [384|128];
            # j=2 -> bank C [256]
            psS0 = psA.tile([P, 512], fp32, tag="psA")
            nc.tensor.matmul(
                psS0, lhsT=kh[:, 0:P], rhs=qh[:, 0:], start=True, stop=True
            )
            psS13 = psA.tile([P, 512], fp32, tag="psA")
            nc.tensor.matmul(
                psS13[:, 0:384], lhsT=kh[:, P : 2 * P], rhs=qh[:, P:],
                start=True, stop=True,
            )
            nc.tensor.matmul(
                psS13[:, 384:512], lhsT=kh[:, 3 * P : 4 * P], rhs=qh[:, 3 * P :],
                start=True, stop=True,
            )
            psS2 = psA.tile([P, 512], fp32, tag="psA")
            nc.tensor.matmul(
                psS2[:, 0:256], lhsT=kh[:, 2 * P : 3 * P], rhs=qh[:, 2 * P :],
                start=True, stop=True,
            )
            # exp (unnormalized) then multiplicative bias+mask
            PTa = soft.tile([P, 512], bf16, tag="PTa", bufs=3)
            PTb = soft.tile([P, 512], bf16, tag="PTb", bufs=3)
            PTc = soft.tile([P, 256], bf16, tag="PTc", bufs=3)
            for ps_in, pt, w, off in (
                (psS0, PTa, 512, 0),
                (psS13, PTb, 512, 512),
                (psS2, PTc, 256, 1024),
            ):
                pt0 = soft.tile([P, 512], bf16, tag="PT0", bufs=3)
                nc.scalar.activation(out=pt0[:, :w], in_=ps_in[:, :w], func=Act.Exp)
                nc.vector.tensor_mul(
                    out=pt[:, :w], in0=pt0[:, :w], in1=ebt[:, off : off + w]
                )
            return PTa, PTb, PTc

        def attn_av(b, h, pts):
            PTa, PTb, PTc = pts
            po = (h % 2) * D
            mo = h // 2
            # AV: psOt rows 0:64 = unnormalized out, row 64 = denominator l
            psOt = psO.tile([P, 512], fp32, tag="psO")

            def pt_rhs(i, j):
                if j == 0:
                    return PTa[:, i * P : (i + 1) * P]
                if j == 1:
                    return PTb[:, (i - 1) * P : i * P]
                if j == 3:
                    return PTb[:, 384:512]
                return PTc[:, (i - 2) * P : (i - 1) * P]

            for i in range(TB):
                for j in range(i + 1):
                    nc.tensor.matmul(
                        psOt[0 : D + 1, i * P : (i + 1) * P],
                        lhsT=vaug[:, b * TB + j, h * (D + 1) : (h + 1) * (D + 1)],
                        rhs=pt_rhs(i, j),
                        start=(j == 0),
                        stop=(j == i),
                    )
            # normalize: r = 1/l; rank-1 matmul broadcasts r into rows 64:128
            # of the same PSUM bank; ACT stages it to SBUF (DVE tensor ops
            # cannot read two PSUM operands); one multiply writes oT
            r_sb = small.tile([1, T], bf16, tag="r", bufs=2)
            with nc.allow_low_precision(reason="bf16 softmax denominators"):
                nc.vector.reciprocal(out=r_sb, in_=psOt[D : D + 1, :])
            nc.tensor.matmul(
                psOt[D : 2 * D, :], lhsT=ones_sb, rhs=r_sb, start=True, stop=True
            )
            rb = soft.tile([D, T], bf16, tag="rb", bufs=2)
            nc.scalar.activation(out=rb, in_=psOt[D : 2 * D, :], func=Act.Copy)
            nc.vector.tensor_mul(
                out=oT_bf[po : po + D, mo, b * T : (b + 1) * T],
                in0=psOt[0:D, :],
                in1=rb,
            )

        # ----- output projection -> y^T [c_out, t] (bias per-partition) ---
        def out_proj(tch, co, defer_store=None):
            ps = psA.tile([P, 512], fp32, tag="psP", bufs=2)
            for ko in range(KO):
                nc.tensor.matmul(
                    ps,
                    lhsT=wo_bf[:, ko, co * P : (co + 1) * P],
                    rhs=oT_bf[:, ko, tch * 512 : (tch + 1) * 512],
                    start=(ko == 0),
                    stop=(ko == KO - 1),
                )
            if defer_store is not None:
                ysb = ypool.tile([P, 512], bf16, tag="ydef", bufs=KO)
            else:
                ysb = ypool.tile([P, 512], bf16, tag="y", bufs=3)
            nc.scalar.activation(
                out=ysb, in_=ps, func=Act.Identity, bias=bo_sb[:, co : co + 1]
            )
            # stores go on the ACT queue (SP must stay clear so the next
            # iteration's input loads can prefetch).  In the final tch=1
            # batch the stores are deferred until all bias-adds have run, so
            # store configs don't delay the PSUM frees that gate the next
            # iteration's first matmuls.
            if defer_store is not None:
                defer_store.append((tch, co, ysb))
            else:
                nc.scalar.dma_start(
                    out=yT[co * P : (co + 1) * P, tch * 512 : (tch + 1) * 512],
                    in_=ysb,
                )

        # ----- one full iteration body: software-pipelined attention (S one
        # head ahead of AV) with V(seq1)/out-proj(seq0) groups interleaved --
        def emit_body():
            emit_loads()
            for mo in range(KO):
                qk_proj(mo)
            for to in range(TB):  # V for seq 0
                v_proj(to)

            # flat schedule of (b, h) attention in order, with filler work
            heads = [(0, h) for h in range(H)] + [(1, h) for h in range(H)]
            filler = {}  # index after which to emit filler group
            for h in range(H):
                if h % 2 == 1 and h // 2 < TB:
                    filler[h] = ("v", TB + h // 2)
                if h % 2 == 1:
                    filler[H + h] = ("o", h // 2)
            pending = None  # (b, h, pts) with S emitted, AV not yet
            for idx, (b, h) in enumerate(heads):
                pts = attn_s(b, h)
                if pending is not None:
                    attn_av(*pending)
                pending = (b, h, pts)
                f = filler.get(idx)
                if f is not None:
                    if f[0] == "v":
                        v_proj(f[1])
                    else:
                        out_proj(0, f[1])
            attn_av(*pending)
            deferred = []
            for co in range(KO):
                out_proj(1, co, defer_store=deferred)
            for tch, co, ysb in deferred:
                nc.scalar.dma_start(
                    out=yT[co * P : (co + 1) * P, tch * 512 : (tch + 1) * 512],
                    in_=ysb,
                )
        # Unroll U iterations per For_i trip: the loop's all-engine drain
        # barrier fires once per U bodies, and within a trip consecutive
        # bodies overlap through the normal tile dependency tracking.
        U = 1
        for cand in (4, 2):
            if reps % cand == 0 and reps >= cand:
                U = cand
                break
        trips = reps // U
        if trips > 1:
            with tc.For_i(0, trips, 1):
                for _ in range(U):
                    emit_body()
        else:
            for _ in range(reps):
                emit_body()

    if split_waits:
        _split_big_waits(nc, mybir, limit=1)
    return nc


def make_in_maps(inputs):
    import ml_dtypes

    bf16 = ml_dtypes.bfloat16
    x = np.asarray(inputs["x"], dtype=np.float32)
    s = np.float32(1.0 / np.sqrt(D))
    wT = {}
    for k in "qkvo":
        w = np.asarray(inputs[f"W{k}"], dtype=np.float32)
        if k == "q":
            w = w * s
        wT[k] = np.ascontiguousarray(w.T).astype(bf16)
    bq = np.asarray(inputs["bq"], dtype=np.float32) * s
    bk = np.asarray(inputs["bk"], dtype=np.float32)
    bo = np.asarray(inputs["bo"], dtype=np.float32) + (
        np.asarray(inputs["Wo"], dtype=np.float32)
        @ np.asarray(inputs["bv"], dtype=np.float32)
    )

    bm = np.asarray(inputs["rel_pos_bias"], dtype=np.float32)[:, :T, :T].copy()
    iu = np.triu_indices(T, 1)
    bm[:, iu[0], iu[1]] = NEG
    # multiplicative form: exp(S+bias) = exp(S) * exp(bias); causal mask
    # becomes an exact multiplicative zero. Transposed to [h, s, t], then
    # the 10 causal blocks packed to [h, 128, 1280] in order j=0,1,3,2.
    bmT = np.exp(bm.transpose(0, 2, 1))
    packed = np.zeros((H, P, BIAS_W), dtype=np.float32)
    for j in range(TB):
        w = T - j * P
        off = BIAS_OFF[j]
        packed[:, :, off : off + w] = bmT[:, j * P : (j + 1) * P, j * P :]
    packed = np.ascontiguousarray(packed.astype(bf16))

    xT_all = x.reshape(N_CORES, NT, C).transpose(0, 2, 1)
    in_maps = []
    for c in range(N_CORES):
        in_maps.append(
            {
                "xT": np.ascontiguousarray(xT_all[c]).astype(bf16),
                "wqT": wT["q"],
                "wkT": wT["k"],
                "wvT": wT["v"],
                "woT": wT["o"],
                "bq": bq,
                "bk": bk,
                "bo": bo,
                "expb": packed,
            }
        )
    return in_maps


def build_jitted(nc, n_cores=N_CORES):
    """Build a persistent jitted shard_map executable for `nc` (the
    multi-core path of bass2jax.run_bass_via_pjrt, kept resident so repeat
    kernel() calls skip retracing)."""
    import jax
    from jax.experimental.shard_map import shard_map
    from jax.sharding import Mesh, NamedSharding, PartitionSpec

    from concourse import mybir
    from concourse.bass2jax import (
        _bass_exec_p,
        install_neuronx_cc_hook,
        partition_id_tensor,
    )

    install_neuronx_cc_hook()
    partition_name = nc.partition_id_tensor.name if nc.partition_id_tensor else None

    in_names, out_names, out_avals, zero_outs = [], [], [], []
    for alloc in nc.m.functions[0].allocations:
        if not isinstance(alloc, mybir.MemoryLocationSet):
            continue
        name = alloc.memorylocations[0].name
        if alloc.kind == "ExternalInput":
            if name != partition_name:
                in_names.append(name)
        elif alloc.kind == "ExternalOutput":
            out_names.append(name)
            shape = tuple(alloc.tensor_shape)
            dtype = mybir.dt.np(alloc.dtype)
            out_avals.append(jax.core.ShapedArray(shape, dtype))
            zero_outs.append(np.zeros(shape, dtype))
    n_params = len(in_names)
    n_outs = len(out_avals)
    all_in_names = list(in_names) + list(out_names)
    if partition_name is not None:
        all_in_names.append(partition_name)
    donate = tuple(range(n_params, n_params + n_outs))

    def _body(*args):
        operands = list(args)
        if partition_name is not None:
            operands.append(partition_id_tensor())
        outs = _bass_exec_p.bind(
            *operands,
            out_avals=tuple(out_avals),
            in_names=tuple(all_in_names),
            out_names=tuple(out_names),
            lowering_input_output_aliases=(),
            sim_require_finite=True,
            sim_require_nnan=True,
            nc=nc,
        )
        return tuple(outs)

    devices = jax.devices()[:n_cores]
    mesh = Mesh(np.asarray(devices), ("core",))
    in_specs = (PartitionSpec("core"),) * (n_params + n_outs)
    out_specs = (PartitionSpec("core"),) * n_outs
    jitted = jax.jit(
        shard_map(_body, mesh=mesh, in_specs=in_specs, out_specs=out_specs,
                  check_rep=False),
        donate_argnums=donate,
        keep_unused=True,
    )
    sharding = NamedSharding(mesh, PartitionSpec("core"))
    return jitted, in_names, out_names, out_avals, zero_outs, sharding


def get_runner():
    """Build the program + executable once; return in_maps -> per-core
    output dicts."""
    if "runner" in _CACHE:
        return _CACHE["runner"]
    import jax

    nc = build_program()
    jitted, in_names, out_names, out_avals, zero_outs, sharding = build_jitted(nc)
    n_cores = N_CORES

    def runner(in_maps):
        concat_in = [
            jax.device_put(
                np.concatenate(
                    [np.asarray(in_maps[c][nm]) for c in range(n_cores)], axis=0
                ),
                sharding,
            )
            for nm in in_names
        ]
        zeros = [
            jax.device_put(
                np.zeros((n_cores * z.shape[0], *z.shape[1:]), z.dtype), sharding
            )
            for z in zero_outs
        ]
        out_arrs = jitted(*concat_in, *zeros)
        return [
            {
                nm: np.asarray(out_arrs[i]).reshape(n_cores, *out_avals[i].shape)[c]
                for i, nm in enumerate(out_names)
            }
            for c in range(n_cores)
        ]

    _CACHE["runner"] = runner
    _CACHE["nc"] = nc
    return runner


def kernel(**inputs) -> np.ndarray:
    runner = get_runner()
    in_maps = make_in_maps(inputs)
    results = runner(in_maps)
    # yT is [C, NT] bf16 per core -> [NT, C] fp32
    out = np.concatenate(
        [
            np.asarray(results[c]["yT"], dtype=np.float32).T.reshape(B_LOC, T, C)
            for c in range(N_CORES)
        ],
        axis=0,
    )
    return out
